# revision 4
# baseline (speedup 1.0000x reference)
"""OCSVM RBF-kernel scoring on Trainium2, data-parallel across 8 NeuronCores.

score[b] = sum_s c[s] * exp(-gamma * ||x_b - s_s||^2) - rho

Rewritten as:
    w[s]  = c[s] * exp(-gamma * s2[s])           (s2 = row norms of support vectors)
    E[b,s]= exp(2*gamma*cross[b,s] - gamma*x2[b]) (cross = X @ S^T, x2 = row norms of X)
    score = sum_s w[s] * E[b,s] - rho

Per-core layout (batch sharded 8 ways, B_loc=2048):
  - X^T and S^T produced on-chip via PE transposes (fp32 has no DMA transpose)
  - cross via fp32r matmuls (full-rate fp32 streaming), K=512 in 4 chunks of 128
  - exp on ScalarE with per-partition scale=2*gamma, bias=-gamma*x2[b]
  - weighted reduction over s on VectorE (scalar_tensor_tensor accum_out),
    with w broadcast to all 128 partitions via a DRAM bounce
"""

import numpy as np

B_TOT = 16384
B_LOC = 2048
S_TOT = 8192
F = 512
P = 128
N_CORES = 8

FC = F // P            # 4 contraction chunks
NB = B_LOC // P        # 16 batch tiles per core
SUPER = 2048           # s-columns per super-tile kept resident in SBUF
N_SUP = S_TOT // SUPER  # 4
NBLK = SUPER // P      # 16 s row-blocks per super
NT = 512               # matmul moving free dim
EW = 1024              # elementwise (exp / reduce) tile width = 2 PSUM banks
N_EW = SUPER // EW     # 2
PK = N_SUP * N_EW      # partial columns per batch tile (8)

_CACHE = {}


def _build():
    """Trace + compile the SPMD Bass program (cached)."""
    if "nc" in _CACHE:
        return _CACHE["nc"]

    from contextlib import ExitStack

    import concourse.bass as bass
    import concourse.mybir as mybir
    import concourse.tile as tile
    from concourse import bacc
    from concourse.masks import make_identity

    f32 = mybir.dt.float32
    f32r = mybir.dt.float32r
    bf16 = mybir.dt.bfloat16
    FT = mybir.ActivationFunctionType
    OP = mybir.AluOpType

    nc = bacc.Bacc("TRN2", target_bir_lowering=False, debug=False)

    x_d = nc.dram_tensor("x", [B_LOC, F], f32, kind="ExternalInput").ap()
    s_d = nc.dram_tensor("s", [S_TOT, F], f32, kind="ExternalInput").ap()
    c_d = nc.dram_tensor("c", [1, S_TOT], f32, kind="ExternalInput").ap()
    rho_d = nc.dram_tensor("rho", [1, 1], f32, kind="ExternalInput").ap()
    gam_d = nc.dram_tensor("gamma", [1, 1], f32, kind="ExternalInput").ap()
    out_d = nc.dram_tensor("out", [NB, P], f32, kind="ExternalOutput").ap()
    # bounce buffer to replicate the s-weights row across all 128 partitions
    wrow_d = nc.dram_tensor("w_bounce", [1, S_TOT], bf16).ap()

    with tile.TileContext(nc) as tc, ExitStack() as ctx:
        const_p = ctx.enter_context(tc.tile_pool(name="const", bufs=1))
        fin_p = ctx.enter_context(tc.tile_pool(name="fin", bufs=1))
        xn_p = ctx.enter_context(tc.tile_pool(name="xn", bufs=3))
        sn_p = ctx.enter_context(tc.tile_pool(name="sn", bufs=4))
        sq_p = ctx.enter_context(tc.tile_pool(name="sq", bufs=2))
        xt_p = ctx.enter_context(tc.tile_pool(name="xt", bufs=1))
        st_p = ctx.enter_context(tc.tile_pool(name="st", bufs=2))
        w_p = ctx.enter_context(tc.tile_pool(name="w", bufs=1))
        row_p = ctx.enter_context(tc.tile_pool(name="row", bufs=1))
        e_p = ctx.enter_context(tc.tile_pool(name="e", bufs=4))
        scr_p = ctx.enter_context(tc.tile_pool(name="scr", bufs=2))
        ps_tr = ctx.enter_context(tc.tile_pool(name="ps_tr", bufs=2, space="PSUM"))
        ps_mm = ctx.enter_context(tc.tile_pool(name="ps_mm", bufs=3, space="PSUM"))

        # ---- constants ----
        ident = const_p.tile([P, P], f32)
        make_identity(nc, ident[:])
        gb = const_p.tile([P, 1], f32)
        nc.sync.dma_start(out=gb[:], in_=gam_d.partition_broadcast(P))
        rb = const_p.tile([P, 1], f32)
        nc.sync.dma_start(out=rb[:], in_=rho_d.partition_broadcast(P))
        two_g = const_p.tile([P, 1], f32)
        nc.scalar.mul(two_g[:], gb[:], 2.0)
        ng = const_p.tile([P, 1], f32)
        nc.scalar.mul(ng[:], gb[:], -1.0)

        x2_pt = fin_p.tile([P, NB], f32)
        bias_pt = fin_p.tile([P, NB], f32)
        parts = fin_p.tile([P, NB * PK], f32)
        score = fin_p.tile([P, NB], f32)

        xt = xt_p.tile([P, FC, B_LOC], f32r)  # X^T, all 4 f-chunks
        w_bc = w_p.tile([P, S_TOT], bf16)     # w replicated across partitions

        # ---- X stage: load, row-norms, transpose ----
        for t in range(NB):
            xn = xn_p.tile([P, F], f32)
            nc.sync.dma_start(out=xn[:], in_=x_d[t * P:(t + 1) * P, :])
            xsq = sq_p.tile([P, F], f32, tag="sq")
            nc.vector.scalar_tensor_tensor(
                out=xsq[:], in0=xn[:], scalar=1.0, in1=xn[:],
                op0=OP.mult, op1=OP.mult, accum_out=x2_pt[:, t:t + 1])
            pt = ps_tr.tile([P, FC, P], f32, tag="pt")
            for fc in range(FC):
                nc.tensor.transpose(pt[:, fc], xn[:, fc * P:(fc + 1) * P], ident[:])
            nc.vector.tensor_copy(out=xt[:, :, t * P:(t + 1) * P], in_=pt[:])
        nc.vector.tensor_scalar_mul(bias_pt[:], x2_pt[:], ng[:])

        # ---- main loop over s super-tiles ----
        for u in range(N_SUP):
            st = st_p.tile([P, FC, SUPER], f32r, tag="st")
            s2_pt = row_p.tile([P, NBLK], f32, tag="s2pt")
            for j in range(NBLK):
                q = u * NBLK + j
                sn = sn_p.tile([P, F], f32)
                nc.sync.dma_start(out=sn[:], in_=s_d[q * P:(q + 1) * P, :])
                ssq = sq_p.tile([P, F], f32, tag="sq")
                nc.vector.scalar_tensor_tensor(
                    out=ssq[:], in0=sn[:], scalar=1.0, in1=sn[:],
                    op0=OP.mult, op1=OP.mult, accum_out=s2_pt[:, j:j + 1])
                pt = ps_tr.tile([P, FC, P], f32, tag="pt")
                for fc in range(FC):
                    nc.tensor.transpose(pt[:, fc], sn[:, fc * P:(fc + 1) * P], ident[:])
                nc.vector.tensor_copy(out=st[:, :, j * P:(j + 1) * P], in_=pt[:])

            # w chain for this super-tile: w[s] = c[s]*exp(-gamma*s2[s]) on one row,
            # then replicate to 128 partitions via DRAM bounce.
            s2t_ps = ps_tr.tile([NBLK, P], f32, tag="pt")
            nc.tensor.transpose(s2t_ps[:], s2_pt[:], ident[:])
            s2_rows = row_p.tile([NBLK, P], f32, tag="s2rows")
            nc.vector.tensor_copy(out=s2_rows[:], in_=s2t_ps[:])
            s2_row = row_p.tile([1, SUPER], f32, tag="s2row")
            nc.sync.dma_start(out=s2_row[:], in_=s2_rows[:])
            c_sl = row_p.tile([1, SUPER], f32, tag="csl")
            nc.sync.dma_start(out=c_sl[:], in_=c_d[:, u * SUPER:(u + 1) * SUPER])
            w_exp = row_p.tile([1, SUPER], f32, tag="wexp")
            nc.scalar.activation(out=w_exp[:], in_=s2_row[:], func=FT.Exp,
                                 scale=ng[:1, :])
            w_sl = row_p.tile([1, SUPER], bf16, tag="wsl")
            nc.vector.tensor_mul(w_sl[:], w_exp[:], c_sl[:])
            nc.sync.dma_start(out=wrow_d[:, u * SUPER:(u + 1) * SUPER], in_=w_sl[:])
            nc.sync.dma_start(
                out=w_bc[:, u * SUPER:(u + 1) * SUPER],
                in_=wrow_d[:, u * SUPER:(u + 1) * SUPER].partition_broadcast(P))

            # matmuls (fp32r full-rate) + exp + weighted reduce
            for t in range(NB):
                for e in range(N_EW):
                    pm = ps_mm.tile([P, EW], f32, tag="pm")
                    for h in range(2):
                        n0 = (e * 2 + h) * NT
                        for fc in range(FC):
                            nc.tensor.matmul(
                                pm[:, h * NT:(h + 1) * NT],
                                xt[:, fc, t * P:(t + 1) * P],
                                st[:, fc, n0:n0 + NT],
                                start=(fc == 0), stop=(fc == FC - 1))
                    et = e_p.tile([P, EW], bf16, tag="et")
                    nc.scalar.activation(out=et[:], in_=pm[:], func=FT.Exp,
                                         scale=two_g[:], bias=bias_pt[:, t:t + 1])
                    dead = scr_p.tile([P, EW], bf16, tag="dead")
                    col = t * PK + u * N_EW + e
                    nc.vector.scalar_tensor_tensor(
                        out=dead[:], in0=et[:], scalar=1.0,
                        in1=w_bc[:, (u * N_EW + e) * EW:(u * N_EW + e + 1) * EW],
                        op0=OP.mult, op1=OP.mult,
                        accum_out=parts[:, col:col + 1])

        # ---- finale: reduce partials, subtract rho, transpose out ----
        pv = parts[:].rearrange("p (t k) -> p t k", k=PK)
        nc.vector.tensor_reduce(out=score[:], in_=pv,
                                axis=mybir.AxisListType.X, op=OP.add)
        nc.vector.tensor_scalar_sub(score[:], score[:], rb[:])
        sc_ps = ps_tr.tile([NB, P], f32, tag="pt")
        nc.tensor.transpose(sc_ps[:], score[:], ident[:])
        sc_t = fin_p.tile([NB, P], f32)
        nc.vector.tensor_copy(out=sc_t[:], in_=sc_ps[:])
        nc.sync.dma_start(out=out_d, in_=sc_t[:])

    nc.compile()
    _CACHE["nc"] = nc
    return nc


def _in_maps(inputs, support_vectors, coefficients, rho, gamma):
    x = np.ascontiguousarray(np.asarray(inputs, dtype=np.float32))
    s = np.ascontiguousarray(np.asarray(support_vectors, dtype=np.float32))
    c = np.ascontiguousarray(np.asarray(coefficients, dtype=np.float32)).reshape(1, S_TOT)
    r = np.asarray(rho, dtype=np.float32).reshape(1, 1)
    g = np.asarray(gamma, dtype=np.float32).reshape(1, 1)
    return [
        {
            "x": x[cid * B_LOC:(cid + 1) * B_LOC],
            "s": s,
            "c": c,
            "rho": r,
            "gamma": g,
        }
        for cid in range(N_CORES)
    ]


def kernel(inputs, support_vectors, coefficients, rho, gamma, _trace=False):
    from concourse.bass_utils import run_bass_kernel_spmd

    nc = _build()
    maps = _in_maps(inputs, support_vectors, coefficients, rho, gamma)
    res = run_bass_kernel_spmd(nc, maps, core_ids=list(range(N_CORES)),
                               trace=_trace)
    out = np.concatenate([np.asarray(r["out"], dtype=np.float32).reshape(B_LOC)
                          for r in res.results])
    if _trace:
        kernel.last_results = res
    return out
